# revision 9
# baseline (speedup 1.0000x reference)
"""Causal depthwise conv1d (B=4, T=8192, F=1024, K=4) on 8 trn2 NeuronCores.

Sharding: feature dim F split 8 ways (128 channels/core, no communication).
Host side transposes each shard to channel-major (128, B*T) and converts to
fp16, halving HBM traffic in both directions (per-core roofline 16.8 MB
@ 358 GB/s ~= 47 us vs 94 us for fp32). The conv is computed at fp16 input
precision with fp32 accumulation (PSUM / DVE / ACT internal), well inside
the 2e-2 harness gate. The bias is added on the host (exact, fp32) after
upconverting the fp16 device output.

On-core layout: partition = channel, free dim = time. Per tile (tcols time
steps + 3-col left halo), out[:, t] = sum_k w_k*x[t+k-3]. Columns split:

  PE chunks (512 cols each), TAP-MAJOR: for k in taps: for c in chunks:
      psum[c] += diag(w_k) @ x_k[c]. Same stationary weight for a whole
      sweep of chunks, so LDWEIGHTS amortizes and consecutive matmuls
      pipeline (tap-minor order measured 379 ns/MM from LDWEIGHTS
      serialization; tap-major targets ~215). PSUM tiles ping-pong
      (bufs=2) so eviction overlaps the next tile's matmul stream.
      Evicted PSUM->SBUF fp32->fp16 alternately by ACT (activation) and
      DVE (tensor_copy) so neither engine saturates.

  Tree unit (tail of each tile): odd taps on ACT (alignment-free
      per-partition scale), even taps on DVE tensor_scalar (4-byte
      aligned + fp16 => 4x packing; scalar_tensor_tensor measured 1x so
      is avoided), combined with three 2x tensor_tensor adds:
        DVE: p0 = w0*x0 ; p2 = w2*x2        (tensor_scalar, 4x)
        ACT: a  = w1*x1 ; c  = w3*x3        (Copy, scale=w)
        DVE: p0 += a ; p2 += c ; out = p0 + p2   (tensor_tensor, 2x)

GpSimd is deliberately unused: any Pool elementwise op contends with DVE's
second SBUF port (measured 3x mutual slowdown). x-loads issue on the Sync
HWDGE ring, out-stores on the ACT ring (qActDynamicHW) so a store waiting
on compute never blocks the next x-load.
"""

import numpy as np
from contextlib import ExitStack

import concourse.bacc as bacc
import concourse.tile as tile
from concourse import mybir
from concourse.bass_utils import run_bass_kernel_spmd

B, T, F, K = 4, 8192, 1024, 4
N_CORES = 8
CPC = F // N_CORES  # 128 channels per core

F16 = mybir.dt.float16
F32 = mybir.dt.float32
MM_N = 512  # moving-operand free dim = one PSUM bank (512 fp32)


def _build_nc(
    n_segs: int,
    seg_cols: int,
    tiles_per_seg: int,
    tree_chunks: int = 1,
):
    nc = bacc.Bacc(
        "TRN2", target_bir_lowering=False, debug=False, num_devices=N_CORES
    )
    tot = n_segs * seg_cols
    tcols = seg_cols // tiles_per_seg
    assert seg_cols % tiles_per_seg == 0
    assert tcols % MM_N == 0
    chunks_per_tile = tcols // MM_N
    assert 0 <= tree_chunks < chunks_per_tile
    pe_chunks = chunks_per_tile - tree_chunks
    assert pe_chunks * MM_N <= 2048, "psum ping-pong needs <= 4 banks/tile"

    H = K - 1  # halo
    # x is host-padded: each batch segment is [H zero cols][seg_cols x cols]
    # so every tile load is one uniform (tcols+H)-wide DMA — no memset, no
    # offset-write APs (a batch-start DMA into xt[:, H:] raced its consumer
    # matmul on HW: stale first columns on straggler partitions).
    x_d = nc.dram_tensor(
        "x", [CPC, n_segs * (seg_cols + H)], F16, kind="ExternalInput"
    ).ap()
    w_d = nc.dram_tensor("w", [CPC, K], F32, kind="ExternalInput").ap()
    dw_d = nc.dram_tensor("dw", [K, CPC, CPC], F16, kind="ExternalInput").ap()
    o_d = nc.dram_tensor("out", [CPC, tot], F16, kind="ExternalOutput").ap()

    mult = mybir.AluOpType.mult
    ident = mybir.ActivationFunctionType.Identity
    copyf = mybir.ActivationFunctionType.Copy

    with tile.TileContext(nc) as tc, ExitStack() as ctx:
        cpool = ctx.enter_context(tc.tile_pool(name="consts", bufs=1))
        # one DMA for all K diagonal matrices: [128, K*128] fp16
        dw_all = cpool.tile([CPC, K * CPC], F16)
        nc.sync.dma_start(
            out=dw_all[:].rearrange("p (k c) -> p k c", k=K),
            in_=dw_d.transpose([1, 0, 2]),
        )
        dw_sb = [dw_all[:, k * CPC : (k + 1) * CPC] for k in range(K)]
        w_sb = cpool.tile([CPC, K], F32)
        nc.sync.dma_start(out=w_sb[:], in_=w_d[:, :])

        xp = ctx.enter_context(tc.tile_pool(name="xp", bufs=4))
        op = ctx.enter_context(tc.tile_pool(name="op", bufs=4))
        t0p = ctx.enter_context(tc.tile_pool(name="t0p", bufs=3))
        t2p = ctx.enter_context(tc.tile_pool(name="t2p", bufs=3))
        tap = ctx.enter_context(tc.tile_pool(name="tap", bufs=3))
        tcp = ctx.enter_context(tc.tile_pool(name="tcp", bufs=3))
        pp = ctx.enter_context(tc.tile_pool(name="pp", bufs=2, space="PSUM"))

        rr = [0]  # round-robin ACT/DVE eviction across tiles

        def emit_tile(t0: int, xsrc: int, ncols: int, n_tree: int):
            # xsrc: column in the padded x_d where this tile's halo starts
            xt = xp.tile([CPC, ncols + H], F16, name=f"xt{t0}", tag="xt")
            nc.sync.dma_start(out=xt[:], in_=x_d[:, xsrc : xsrc + ncols + H])

            ot = op.tile([CPC, ncols], F16, name=f"ot{t0}", tag="ot")

            n_pe = ncols // MM_N - n_tree
            pe_cols = n_pe * MM_N
            if n_pe > 0:
                ps = pp.tile([CPC, pe_cols], F32, name=f"ps{t0}", tag="ps")
                for k in range(K):
                    for c in range(n_pe):
                        c0 = c * MM_N
                        nc.tensor.matmul(
                            ps[:, c0 : c0 + MM_N],
                            dw_sb[k][:],
                            xt[:, c0 + k : c0 + k + MM_N],
                            start=(k == 0),
                            stop=(k == K - 1),
                            skip_group_check=True,
                        )
                if rr[0] % 2 == 0:
                    nc.scalar.activation(
                        ot[:, 0:pe_cols], ps[:], ident, bias=0.0, scale=1.0
                    )
                else:
                    nc.vector.tensor_copy(ot[:, 0:pe_cols], ps[:])
                rr[0] += 1

            if n_tree > 0:
                q = pe_cols
                n = n_tree * MM_N
                p0 = t0p.tile([CPC, n], F16, name=f"p0_{t0}", tag="p0")
                p2 = t2p.tile([CPC, n], F16, name=f"p2_{t0}", tag="p2")
                a = tap.tile([CPC, n], F16, name=f"a{t0}", tag="a")
                c_ = tcp.tile([CPC, n], F16, name=f"c{t0}", tag="c")
                nc.vector.tensor_scalar(
                    p0[:], xt[:, q : q + n], w_sb[:, 0:1], None, mult
                )
                nc.scalar.activation(
                    a[:], xt[:, q + 1 : q + 1 + n],
                    copyf, bias=0.0, scale=w_sb[:, 1:2],
                )
                nc.vector.tensor_scalar(
                    p2[:], xt[:, q + 2 : q + 2 + n], w_sb[:, 2:3], None, mult
                )
                nc.scalar.activation(
                    c_[:], xt[:, q + 3 : q + 3 + n],
                    copyf, bias=0.0, scale=w_sb[:, 3:4],
                )
                nc.vector.tensor_add(p0[:], p0[:], a[:])
                nc.vector.tensor_add(p2[:], p2[:], c_[:])
                nc.vector.tensor_add(ot[:, q : q + n], p0[:], p2[:])

            # out-stores issue from the ACT HWDGE ring (qActDynamicHW) so a
            # store waiting on compute never blocks the Sync ring's x-loads
            nc.scalar.dma_start(out=o_d[:, t0 : t0 + ncols], in_=ot[:])

        for s in range(n_segs):
            for j in range(tiles_per_seg):
                t0 = s * seg_cols + j * tcols
                xsrc = s * (seg_cols + H) + j * tcols
                emit_tile(t0, xsrc, tcols, tree_chunks)

    nc.compile()
    return nc


def _shard_inputs(x, w):
    # x: (B, T, F) -> channel-major fp16 with a (K-1)-col zero pad before
    # each batch segment: (F, B*(T+K-1)).
    H = K - 1
    xs = np.zeros((F, B * (T + H)), np.float16)
    xt = np.transpose(x, (2, 0, 1)).astype(np.float16)  # (F, B, T)
    for s in range(B):
        xs[:, s * (T + H) + H : (s + 1) * (T + H)] = xt[:, s, :]
    in_maps = []
    for cix in range(N_CORES):
        sl = slice(cix * CPC, (cix + 1) * CPC)
        wc = np.ascontiguousarray(w[:, 0, sl])  # (K, CPC) fp32
        dw = np.zeros((K, CPC, CPC), np.float16)
        for k in range(K):
            np.fill_diagonal(dw[k], wc[k].astype(np.float16))
        in_maps.append(
            {
                "x": np.ascontiguousarray(xs[sl]),
                "w": np.ascontiguousarray(wc.T),
                "dw": dw,
            }
        )
    return in_maps


def _unshard_output(results, b) -> np.ndarray:
    out = np.empty((B, T, F), np.float32)
    for cix in range(N_CORES):
        oc = results[cix]["out"]  # (CPC, B*T) fp16
        out[:, :, cix * CPC : (cix + 1) * CPC] = (
            oc.astype(np.float32).reshape(CPC, B, T).transpose(1, 2, 0)
        )
    if np.any(b):
        out += b.astype(np.float32)
    return out


def _run(
    x,
    w,
    b,
    trace: bool = False,
    tiles_per_seg: int = 4,
    tree_chunks: int = 1,
    tmpdir=None,
):
    x = np.asarray(x, dtype=np.float32)
    w = np.asarray(w, dtype=np.float32)
    b = np.asarray(b, dtype=np.float32)
    in_maps = _shard_inputs(x, w)
    nc = _build_nc(B, T, tiles_per_seg, tree_chunks=tree_chunks)
    br = run_bass_kernel_spmd(
        nc, in_maps, core_ids=list(range(N_CORES)), trace=trace, tmpdir=tmpdir
    )
    return _unshard_output(br.results, b), br


def kernel(x, w, b):
    out, _ = _run(x, w, b, trace=False)
    return out


# revision 13
# speedup vs baseline: 1.2491x; 1.2491x over previous
"""Causal depthwise conv1d (B=4, T=8192, F=1024, K=4) on 8 trn2 NeuronCores.

Sharding: feature dim F split 8 ways (128 channels/core, no communication).
Host side transposes each shard to channel-major (128, B*T) and converts to
fp16, halving HBM traffic in both directions (per-core roofline 16.8 MB
@ 358 GB/s ~= 47 us vs 94 us for fp32). The conv is computed at fp16 input
precision with fp32 accumulation (PSUM / DVE / ACT internal), well inside
the 2e-2 harness gate. The bias is added on the host (exact, fp32) after
upconverting the fp16 device output.

On-core layout: partition = channel, free dim = time. Per tile (tcols time
steps + 3-col left halo), out[:, t] = sum_k w_k*x[t+k-3]. Columns split:

  PE chunks (512 cols each), TAP-MAJOR: for k in taps: for c in chunks:
      psum[c] += diag(w_k) @ x_k[c]. Same stationary weight for a whole
      sweep of chunks, so LDWEIGHTS amortizes and consecutive matmuls
      pipeline (tap-minor order measured 379 ns/MM from LDWEIGHTS
      serialization; tap-major targets ~215). PSUM tiles ping-pong
      (bufs=2) so eviction overlaps the next tile's matmul stream.
      Evicted PSUM->SBUF fp32->fp16 alternately by ACT (activation) and
      DVE (tensor_copy) so neither engine saturates.

  Tree unit (tail of each tile): odd taps on ACT (alignment-free
      per-partition scale), even taps on DVE tensor_scalar (4-byte
      aligned + fp16 => 4x packing; scalar_tensor_tensor measured 1x so
      is avoided), combined with three 2x tensor_tensor adds:
        DVE: p0 = w0*x0 ; p2 = w2*x2        (tensor_scalar, 4x)
        ACT: a  = w1*x1 ; c  = w3*x3        (Copy, scale=w)
        DVE: p0 += a ; p2 += c ; out = p0 + p2   (tensor_tensor, 2x)

GpSimd is deliberately unused: any Pool elementwise op contends with DVE's
second SBUF port (measured 3x mutual slowdown). x-loads issue on the Sync
HWDGE ring, out-stores on the ACT ring (qActDynamicHW) so a store waiting
on compute never blocks the next x-load.
"""

import numpy as np
from contextlib import ExitStack

import concourse.bacc as bacc
import concourse.tile as tile
from concourse import mybir
from concourse.bass_utils import run_bass_kernel_spmd

B, T, F, K = 4, 8192, 1024, 4
N_CORES = 8
CPC = F // N_CORES  # 128 channels per core

F16 = mybir.dt.float16
F32 = mybir.dt.float32
MM_N = 512  # moving-operand free dim = one PSUM bank (512 fp32)


def _build_nc(
    n_segs: int,
    seg_cols: int,
    tiles_per_seg: int,
    tree_sched=(1, 2, 1, 2),
):
    nc = bacc.Bacc(
        "TRN2", target_bir_lowering=False, debug=False, num_devices=N_CORES
    )
    tot = n_segs * seg_cols
    tcols = seg_cols // tiles_per_seg
    assert seg_cols % tiles_per_seg == 0
    assert tcols % MM_N == 0
    chunks_per_tile = tcols // MM_N
    assert all(0 <= t < chunks_per_tile for t in tree_sched)
    assert all(
        (chunks_per_tile - t) * MM_N <= 2048 for t in tree_sched
    ), "psum ping-pong needs <= 4 banks/tile"

    H = K - 1  # halo
    # x is host-padded: each batch segment is [H zero cols][seg_cols x cols]
    # so every tile load is one uniform (tcols+H)-wide DMA — no memset, no
    # offset-write APs (a batch-start DMA into xt[:, H:] raced its consumer
    # matmul on HW: stale first columns on straggler partitions).
    x_d = nc.dram_tensor(
        "x", [CPC, n_segs * (seg_cols + H)], F16, kind="ExternalInput"
    ).ap()
    w_d = nc.dram_tensor("w", [CPC, K], F32, kind="ExternalInput").ap()
    dw_d = nc.dram_tensor("dw", [K, CPC, CPC], F16, kind="ExternalInput").ap()
    o_d = nc.dram_tensor("out", [CPC, tot], F16, kind="ExternalOutput").ap()

    mult = mybir.AluOpType.mult
    add = mybir.AluOpType.add
    ident = mybir.ActivationFunctionType.Identity

    with tile.TileContext(nc) as tc, ExitStack() as ctx:
        cpool = ctx.enter_context(tc.tile_pool(name="consts", bufs=1))
        # one DMA for all K diagonal matrices: [128, K*128] fp16
        dw_all = cpool.tile([CPC, K * CPC], F16)
        nc.sync.dma_start(
            out=dw_all[:].rearrange("p (k c) -> p k c", k=K),
            in_=dw_d.transpose([1, 0, 2]),
        )
        dw_sb = [dw_all[:, k * CPC : (k + 1) * CPC] for k in range(K)]
        w_sb = cpool.tile([CPC, K], F32)
        nc.sync.dma_start(out=w_sb[:], in_=w_d[:, :])

        xp = ctx.enter_context(tc.tile_pool(name="xp", bufs=5))
        op = ctx.enter_context(tc.tile_pool(name="op", bufs=4))
        t0p = ctx.enter_context(tc.tile_pool(name="t0p", bufs=3))
        t2p = ctx.enter_context(tc.tile_pool(name="t2p", bufs=3))
        pp = ctx.enter_context(tc.tile_pool(name="pp", bufs=2, space="PSUM"))

        def emit_tile(t0: int, xsrc: int, ncols: int, n_tree: int):
            # xsrc: column in the padded x_d where this tile's halo starts
            xt = xp.tile([CPC, ncols + H], F16, name=f"xt{t0}", tag="xt")
            nc.sync.dma_start(out=xt[:], in_=x_d[:, xsrc : xsrc + ncols + H])

            ot = op.tile([CPC, ncols], F16, name=f"ot{t0}", tag="ot")

            n_pe = ncols // MM_N - n_tree
            pe_cols = n_pe * MM_N
            if n_pe > 0:
                # tap-minor: contiguous 4-MM accumulation group per chunk
                ps = pp.tile([CPC, pe_cols], F32, name=f"ps{t0}", tag="ps")
                for c in range(n_pe):
                    c0 = c * MM_N
                    for k in range(K):
                        nc.tensor.matmul(
                            ps[:, c0 : c0 + MM_N],
                            dw_sb[k][:],
                            xt[:, c0 + k : c0 + k + MM_N],
                            start=(k == 0),
                            stop=(k == K - 1),
                        )
                nc.scalar.activation(
                    ot[:, 0:pe_cols], ps[:], ident, bias=0.0, scale=1.0
                )

            if n_tree > 0:
                # DVE-only tree: even taps via 4x tensor_scalar, odd taps
                # via scalar_tensor_tensor MACs (1x regardless, so the odd
                # shifts' misalignment costs nothing), one 2x combine.
                q = pe_cols
                n = n_tree * MM_N
                p0 = t0p.tile([CPC, n], F16, name=f"p0_{t0}", tag="p0")
                p2 = t2p.tile([CPC, n], F16, name=f"p2_{t0}", tag="p2")
                nc.vector.tensor_scalar(
                    p0[:], xt[:, q : q + n], w_sb[:, 0:1], None, mult
                )
                nc.vector.scalar_tensor_tensor(
                    p0[:], xt[:, q + 1 : q + 1 + n], w_sb[:, 1:2], p0[:],
                    mult, add,
                )
                nc.vector.tensor_scalar(
                    p2[:], xt[:, q + 2 : q + 2 + n], w_sb[:, 2:3], None, mult
                )
                nc.vector.scalar_tensor_tensor(
                    p2[:], xt[:, q + 3 : q + 3 + n], w_sb[:, 3:4], p2[:],
                    mult, add,
                )
                nc.vector.tensor_add(ot[:, q : q + n], p0[:], p2[:])

            # out-stores issue from the ACT HWDGE ring (qActDynamicHW) right
            # after the eviction so a store never blocks the Sync x-loads
            # and the ACT queue head never waits on the DVE tree.
            nc.scalar.dma_start(out=o_d[:, t0 : t0 + ncols], in_=ot[:])

        for s in range(n_segs):
            for j in range(tiles_per_seg):
                t0 = s * seg_cols + j * tcols
                xsrc = s * (seg_cols + H) + j * tcols
                idx = s * tiles_per_seg + j
                emit_tile(t0, xsrc, tcols, tree_sched[idx % len(tree_sched)])

    nc.compile()
    return nc


def _shard_inputs(x, w):
    # x: (B, T, F) -> channel-major fp16 with a (K-1)-col zero pad before
    # each batch segment: (F, B*(T+K-1)).
    H = K - 1
    xs = np.zeros((F, B * (T + H)), np.float16)
    xt = np.transpose(x, (2, 0, 1)).astype(np.float16)  # (F, B, T)
    for s in range(B):
        xs[:, s * (T + H) + H : (s + 1) * (T + H)] = xt[:, s, :]
    in_maps = []
    for cix in range(N_CORES):
        sl = slice(cix * CPC, (cix + 1) * CPC)
        wc = np.ascontiguousarray(w[:, 0, sl])  # (K, CPC) fp32
        dw = np.zeros((K, CPC, CPC), np.float16)
        for k in range(K):
            np.fill_diagonal(dw[k], wc[k].astype(np.float16))
        in_maps.append(
            {
                "x": np.ascontiguousarray(xs[sl]),
                "w": np.ascontiguousarray(wc.T),
                "dw": dw,
            }
        )
    return in_maps


def _unshard_output(results, b) -> np.ndarray:
    out = np.empty((B, T, F), np.float32)
    for cix in range(N_CORES):
        oc = results[cix]["out"]  # (CPC, B*T) fp16
        out[:, :, cix * CPC : (cix + 1) * CPC] = (
            oc.astype(np.float32).reshape(CPC, B, T).transpose(1, 2, 0)
        )
    if np.any(b):
        out += b.astype(np.float32)
    return out


def _run(
    x,
    w,
    b,
    trace: bool = False,
    tiles_per_seg: int = 4,
    tree_sched=(1, 2, 1, 2),
    tmpdir=None,
):
    x = np.asarray(x, dtype=np.float32)
    w = np.asarray(w, dtype=np.float32)
    b = np.asarray(b, dtype=np.float32)
    in_maps = _shard_inputs(x, w)
    nc = _build_nc(B, T, tiles_per_seg, tree_sched=tuple(tree_sched))
    br = run_bass_kernel_spmd(
        nc, in_maps, core_ids=list(range(N_CORES)), trace=trace, tmpdir=tmpdir
    )
    return _unshard_output(br.results, b), br


def kernel(x, w, b):
    out, _ = _run(x, w, b, trace=False)
    return out
